# revision 56
# baseline (speedup 1.0000x reference)
"""Multi-head attention (B=2, S=4096, D=512, H=8) on 8 Trainium2 NeuronCores.

Sharding: core c handles batch b = c // 4 and head-group g = c % 4 (2 heads =
columns/rows [128g : 128g+128] of the projection weights).  Each core runs its
2 heads' attention over the full sequence plus the partial output projection
through the matching 128 rows of Wo; the host sums the 4 partials per batch
(+ bo) in fp32.

v2 schedule: the two heads' QK matmuls have K=64 contraction each, so they
are issued as adjacent row-tiled pairs (tile_position rows 0-63 / 64-127)
that run concurrently in the PE array -- halving QK time vs the zero-padded
K=128 form.  Softmax exp is split between the ACT engine (true exp) and the
DVE (Schraudolph int16-bitcast approximation, rel err +-3%) to unload the
ACT bottleneck.  Head 1's probability tiles are parked in SBUF and its PV
accumulation runs as a burst after head 0's, so one PSUM accumulator bank
serves both heads; PSUM = 4 banks lg(h0, double-buffered) + 2 banks lg(h1)
+ 1 pv + 1 scratch.  Softmax denominators ride the ones-column of vaug;
per-qb denominator rows are staged and PE-transposed into per-partition
reciprocals applied after the output projection.

Numerics: fp16 storage for X/W/q/k/v/P/ctx, fp32 PSUM accumulation, fp32
softmax denominators (no row-max: logits ~N(0,1), exp safe in fp32).
"""

import numpy as np

import concourse.bass as bass
import concourse.tile as tile
from concourse import bacc, mybir
from concourse.bass_utils import run_bass_kernel_spmd
from concourse.masks import make_identity

P = 128
D = 512
GD = 128  # head-group width: 2 heads x 64
HD = 64
S_FULL = 4096
B_FULL = 2
N_CORES = 8
NT = S_FULL // P  # 32 key tiles
QB = S_FULL // 512  # 8 query blocks
NPER = 16  # periods (2-key-tile chunks) per query block
PC = 1024  # piece height for input DMA transposes
F32 = mybir.dt.float32
F16 = mybir.dt.float16
I16 = mybir.dt.int16
EXP = mybir.ActivationFunctionType.Exp
MULT = mybir.AluOpType.mult
ADD = mybir.AluOpType.add

# Schraudolph fp16 exp on DVE: i16 = rint(lg * SCH_A + SCH_B) bitcast to
# fp16 approximates exp(0.125 * lg), rel err in [-3.01%, +3.02%] (DVE
# fp32->int16 converts round-to-nearest; HW-probed).
SCH_A = 0.125 * 1024.0 * 1.4426950408889634
SCH_B = 1024.0 * (15.0 - 0.0436)
# periods whose h1 exp chunk routes to DVE (qb >= 1 only; qb0's DVE queue
# is busy with projection bias-adds and must not stall behind them)
DVE_PERIODS = frozenset((0, 1, 2, 4, 5, 6, 7, 8, 10, 11, 12, 14))


def _emit(tc, io):
    nc = tc.nc
    xq, xk, xv, wq, wk, wv, wo, bq, bk, out = io

    with (
        tc.tile_pool(name="persist", bufs=1) as pp,
        tc.tile_pool(name="lg0p", bufs=2, space="PSUM") as lg0p,
        tc.tile_pool(name="lg1p", bufs=1, space="PSUM") as lg1p,
        tc.tile_pool(name="mpsum", bufs=1, space="PSUM") as mp,
        tc.tile_pool(name="pvp", bufs=1, space="PSUM") as pvp,
        tc.tile_pool(name="xtp", bufs=16) as xtp,
        tc.tile_pool(name="pt0p", bufs=24) as pt0p,
        tc.tile_pool(name="pt1p", bufs=24) as pt1p,
        tc.tile_pool(name="stp", bufs=3) as stp,
        tc.tile_pool(name="obp", bufs=5) as obp,
    ):
        ident16 = pp.tile([P, P], F16, name="ident16")
        make_identity(nc, ident16)
        # weights arrive host-pre-arranged as [P, 4, GD] so the DMA reads
        # contiguous 1KB lines (the strided gather was ~50us on the queue)
        wqs = pp.tile([P, 4, GD], F16, name="wqs")
        wks = pp.tile([P, 4, GD], F16, name="wks")
        wvs = pp.tile([P, 4, GD], F16, name="wvs")
        nc.gpsimd.dma_start(wqs, wq)
        nc.gpsimd.dma_start(wks, wk)
        nc.gpsimd.dma_start(wvs, wv)
        wos = pp.tile([P, D], F16, name="wos")
        nc.gpsimd.dma_start(wos, wo)
        bqs = pp.tile([P, 1], F32, name="bqs")
        bks = pp.tile([P, 1], F32, name="bks")
        nc.gpsimd.dma_start(bqs, bq[:, None])
        nc.gpsimd.dma_start(bks, bk[:, None])

        # big persistent activations (all fp16).  qT merged: rows 0:64 hold
        # head-0 dims, rows 64:128 head-1 dims (= projection output layout).
        kT = pp.tile([P, S_FULL], F16, name="kT")
        qT = pp.tile([P, S_FULL], F16, name="qT")
        vaug0 = pp.tile([P, NT, P], F16, name="vaug0")
        vaug1 = pp.tile([P, NT, P], F16, name="vaug1")
        vaug = [vaug0, vaug1]
        nc.gpsimd.memset(vaug0, 0.0)
        nc.gpsimd.memset(vaug0[:, :, HD : HD + 1], 1.0)
        nc.gpsimd.memset(vaug1, 0.0)
        nc.gpsimd.memset(vaug1[:, :, 0:1], 1.0)
        uctx16 = pp.tile([P, S_FULL], F16, name="uctx16")
        rd = pp.tile([P, NT, 2], F32, name="rd")

        # ------------- input loads (host pre-transposed), 1024-row pieces ---
        # x arrives as [4, 128, S] = x.T tiled over d: plain contiguous DMA
        # at full bandwidth (the on-device xbar transpose streams 64B lines
        # at ~23 GB/s and was pacing the whole first third of the kernel).
        xts = {}

        def emit_piece_dma(which, row0, nrows):
            src = {"k": xk, "v": xv, "q": xq}[which]
            tiles = []
            for dt in range(4):
                xt = xtp.tile([P, PC], F16, tag="xt", name="xt")[:, :nrows]
                nc.sync.dma_start(xt, src[dt, :, row0 : row0 + nrows])
                tiles.append(xt)
            xts[(which, row0)] = tiles

        dma_order = [
            ("k", 0, 512), ("q", 0, 512), ("k", 512, 512),
            ("k", 1024, 1024), ("k", 2048, 1024), ("k", 3072, 1024),
            ("q", 512, 512),
            ("v", 0, 1024), ("v", 1024, 1024), ("v", 2048, 1024),
            ("v", 3072, 1024),
            ("q", 1024, 1024), ("q", 2048, 1024), ("q", 3072, 1024),
        ]
        for which, row0, nrows in dma_order:
            emit_piece_dma(which, row0, nrows)

        # ------------- projections (emitted piecewise via hooks) ------------
        def emit_kq_proj(which, row0, nrows, alt=False):
            w = {"k": wks, "q": wqs}[which]
            dst = {"k": kT, "q": qT}[which]
            bias = {"k": bks, "q": bqs}[which]
            tiles = xts.pop((which, row0))
            for sbl in range(nrows // 512):
                cols = slice(row0 + sbl * 512, row0 + (sbl + 1) * 512)
                lcol = slice(sbl * 512, (sbl + 1) * 512)
                # during qb0 the pv bank is still idle: alternate the psum
                # accumulator so consecutive blocks don't serialize on the
                # DVE bias-add freeing a single bank
                pl, tg = (pvp, "pv") if (alt and sbl % 2) else (mp, "mA")
                acc = pl.tile([P, 512], F32, tag=tg, name="acc")
                for dt in range(4):
                    nc.tensor.matmul(
                        acc,
                        lhsT=w[:, dt, :],
                        rhs=tiles[dt][:, lcol],
                        start=(dt == 0),
                        stop=(dt == 3),
                    )
                nc.vector.tensor_scalar_add(dst[:, cols], acc[:], bias[:])

        def emit_v_chain(row0, nrows, g0=None):
            if g0 is None:
                tiles = xts.pop(("v", row0))
            else:
                tiles = xts[("v", row0)] if g0 == 0 else xts.pop(("v", row0))
            ktiles = nrows // P
            groups = range(ktiles // 4) if g0 is None else [g0]
            for g in groups:
                psv = mp.tile([P, 4, P], F32, tag="mA", name="psv")
                for i in range(4):
                    kl = 4 * g + i
                    lcol = slice(kl * P, (kl + 1) * P)
                    for dt in range(4):
                        nc.tensor.matmul(
                            psv[:, i, :],
                            lhsT=tiles[dt][:, lcol],
                            rhs=wvs[:, dt, :],
                            start=(dt == 0),
                            stop=(dt == 3),
                        )
                kt0 = row0 // P + 4 * g
                nc.vector.tensor_copy(
                    out=vaug0[:, kt0 : kt0 + 4, 0:HD], in_=psv[:, :, 0:HD]
                )
                nc.vector.tensor_copy(
                    out=vaug1[:, kt0 : kt0 + 4, HD:P], in_=psv[:, :, HD:P]
                )

        # ------------------ attention machinery ------------------------------
        pv_tiles = {}
        stage_tiles = {}

        def emit_qk_pair(qb, kt, lg0, lg1, half):
            qcols = slice(qb * 512, (qb + 1) * 512)
            kcols = slice(kt * P, (kt + 1) * P)
            # h1 first (its lg1 ring slot frees latest); h0 lands right
            # behind it in the PE queue so the K=64 pair overlaps in-array.
            nc.tensor.matmul(
                lg1[:, half * 512 : (half + 1) * 512],
                lhsT=kT[HD:P, kcols],
                rhs=qT[HD:P, qcols],
                start=True,
                stop=True,
            )
            nc.tensor.matmul(
                lg0[:, half * 512 : (half + 1) * 512],
                lhsT=kT[0:HD, kcols],
                rhs=qT[0:HD, qcols],
                start=True,
                stop=True,
            )

        def emit_pv(qb, h, kt, ptt, half):
            key = (qb, h)
            if key not in pv_tiles:
                pv_tiles[key] = pvp.tile([P, 512], F32, tag="pv", name="pv")
            nc.tensor.matmul(
                pv_tiles[key],
                lhsT=vaug[h][:, kt, :],
                rhs=ptt[:, half * 512 : (half + 1) * 512],
                start=(kt == 0),
                stop=(kt == NT - 1),
            )

        def emit_tail(qb, h):
            qcols = slice(qb * 512, (qb + 1) * 512)
            pv_acc = pv_tiles.pop((qb, h))
            rows = slice(0, HD) if h == 0 else slice(HD, P)
            drow = HD if h == 0 else 0
            if h == 0:
                stage_tiles[qb] = stp.tile([48, 512], F16, tag="stg", name="stg")
            stg = stage_tiles[qb]
            srow = 32 * h  # DVE partition offsets must be 32-aligned
            nc.vector.tensor_copy(out=uctx16[rows, qcols], in_=pv_acc[rows, :])
            nc.vector.tensor_copy(
                out=stg[srow : srow + 1, :], in_=pv_acc[drow : drow + 1, :]
            )

        # --------- output projection ops (one queue slot each) ---------------
        def c_transpose(qb, heads=None, pop=True):
            def fn():
                stg = stage_tiles.pop(qb) if pop else stage_tiles[qb]
                for sl in range(4):
                    st = 4 * qb + sl
                    tps = mp.tile([P, 33], F16, tag="mA", name="tps")
                    if heads is None:
                        nc.tensor.transpose(
                            tps,
                            stg[0:33, sl * P : (sl + 1) * P],
                            ident16[0:33, 0:33],
                        )
                        nc.vector.reciprocal(rd[:, st, 0:1], tps[:, 0:1])
                        nc.vector.reciprocal(rd[:, st, 1:2], tps[:, 32:33])
                    else:
                        (h,) = heads
                        nc.tensor.transpose(
                            tps[:, 0:1],
                            stg[32 * h : 32 * h + 1, sl * P : (sl + 1) * P],
                            ident16[32 * h : 32 * h + 1, 32 * h : 32 * h + 1],
                        )
                        nc.vector.reciprocal(rd[:, st, h : h + 1], tps[:, 0:1])
            return fn

        ob0_tiles = {}

        def c_mm(qb, sl, half, pool=None, tag=None):
            def fn():
                st = 4 * qb + sl
                stcols = slice(st * P, (st + 1) * P)
                pl, tg = (pool or mp), (tag or "mA")
                ps = pl.tile([P, D], F32, tag=tg, name="cps")
                rows = slice(0, HD) if half == 0 else slice(HD, P)
                nc.tensor.matmul(
                    ps,
                    lhsT=uctx16[rows, stcols],
                    rhs=wos[rows, :],
                    start=True,
                    stop=True,
                )
                if half == 0:
                    ob0 = obp.tile([P, D], F32, tag="ob0", name="ob0")
                    nc.vector.tensor_scalar_mul(ob0, ps[:], rd[:, st, 0:1])
                    ob0_tiles[st] = ob0
                else:
                    ob = obp.tile([P, D], F16, tag="ob", name="ob")
                    nc.vector.scalar_tensor_tensor(
                        out=ob,
                        in0=ps[:],
                        scalar=rd[:, st, 1:2],
                        in1=ob0_tiles.pop(st),
                        op0=MULT,
                        op1=ADD,
                    )
                    nc.sync.dma_start(out[st * P : (st + 1) * P, :], ob)
            return fn

        def c_ops(qb):
            ops = [c_transpose(qb)]
            for sl in range(4):
                ops.append(c_mm(qb, sl, 0))
                ops.append(c_mm(qb, sl, 1))
            return ops

        # ----------------------- schedules -----------------------------------
        hooks = {}

        def at(qb, p, fn):
            hooks.setdefault((qb, p), []).append(fn)

        at(0, 0, lambda: emit_kq_proj("k", 512, 512, alt=True))
        at(0, 1, lambda: emit_kq_proj("k", 1024, 1024, alt=True))
        at(0, 4, lambda: emit_kq_proj("k", 2048, 1024, alt=True))
        at(0, 7, lambda: emit_kq_proj("k", 3072, 1024, alt=True))
        at(0, 11, lambda: emit_kq_proj("q", 512, 512, alt=True))
        at(0, 13, lambda: emit_v_chain(0, 1024, 0))
        at(0, 14, lambda: emit_v_chain(0, 1024, 1))
        at(1, 3, lambda: emit_v_chain(1024, 1024, 0))
        at(1, 4, lambda: emit_v_chain(1024, 1024, 1))
        at(1, 6, lambda: emit_v_chain(2048, 1024, 0))
        at(1, 7, lambda: emit_v_chain(2048, 1024, 1))
        at(1, 9, lambda: emit_v_chain(3072, 1024, 0))
        at(1, 10, lambda: emit_v_chain(3072, 1024, 1))
        at(1, 13, lambda: emit_kq_proj("q", 1024, 1024))
        at(2, 8, lambda: emit_kq_proj("q", 2048, 1024))
        at(3, 5, lambda: emit_kq_proj("q", 3072, 1024))

        # PV work deque: ("pv", qb, h, kt, ptt, half, gate) / ("tail", qb, h,
        # None).  Entries are appended in global emission order (per qb: all
        # h0, tail0, all parked h1, tail1); drain_pv emits the head entry
        # only once its gate period has been reached (gates hold qb0's PV
        # until the v-chains have produced vaug).
        pv_deque = []
        h1_parked = []
        side_queue = []
        # qb0 h0 PV gate per 8-kt group (after the matching v-chain hooks)
        qb0_gate = {0: (0, 15), 1: (1, 5), 2: (1, 8), 3: (1, 11)}

        def gate_passed(gate, now):
            return gate is None or now is None or gate <= now

        def drain_pv(budget, now):
            # Emit queued PV matmuls, but never one produced in the current
            # period: a PV whose exp was just emitted would sit at the PE
            # FIFO head waiting on ACT/DVE and serialize the whole pipeline.
            done = 0
            while pv_deque and done < budget:
                e = pv_deque[0]
                if not gate_passed(e[-1] if e[0] == "pv" else None, now):
                    break
                if e[0] == "tail":
                    _, qb_, h_, _g = e
                    emit_tail(qb_, h_)
                    pv_deque.pop(0)
                    if qb_ == QB - 1:
                        # last qb: overlap head-0's output projection with
                        # the h1 PV burst instead of waiting for both tails
                        if h_ == 0:
                            side_queue.append(
                                c_transpose(qb_, heads=(0,), pop=False)
                            )
                            side_queue.extend(
                                c_mm(qb_, sl, 0) for sl in range(4)
                            )
                        else:
                            side_queue.append(c_transpose(qb_, heads=(1,)))
                            # pv bank is free after this tail: alternate it
                            # with mp so each matmul overlaps the previous
                            # DVE combine instead of waiting for the bank
                            side_queue.extend(
                                c_mm(
                                    qb_, sl, 1,
                                    pool=(pvp if sl % 2 else None),
                                    tag=("pv" if sl % 2 else None),
                                )
                                for sl in range(4)
                            )
                    elif h_ == 1:
                        side_queue.extend(c_ops(qb_))
                    continue
                _, qb_, h_, kt_, ptt_, half_, _g = e
                if now is not None and (qb_, kt_ // 2) >= now:
                    break
                emit_pv(qb_, h_, kt_, ptt_, half_)
                pv_deque.pop(0)
                done += 1
            return done

        def drain_side(budget):
            done = 0
            while side_queue and done < budget:
                side_queue.pop(0)()
                done += 1

        # ------------------------- main loop ----------------------------------
        emit_kq_proj("k", 0, 512, alt=True)
        emit_kq_proj("q", 0, 512, alt=True)

        def emit_qk_chunk(qb, p):
            lg0 = lg0p.tile([P, 1024], F32, tag="lg0", name="lg0")
            lg1 = lg1p.tile([P, 1024], F32, tag="lg1", name="lg1")
            for half in range(2):
                emit_qk_pair(qb, 2 * p + half, lg0, lg1, half)
            return lg0, lg1

        lg_cur = emit_qk_chunk(0, 0)

        for qb in range(QB):
            for p in range(NPER):
                lg0, lg1 = lg_cur
                # consumers for chunk (qb, p).  h1 drains first: the next
                # QK pair's h1 matmul waits on the single-buffered lg1 ring,
                # so its consumer must not queue behind exp h0 on ACT.
                ptt0 = pt0p.tile([P, 1024], F16, tag="pt0", name="ptt0")
                ptt1 = pt1p.tile([P, 1024], F16, tag="pt1", name="ptt1")
                if qb >= 1 and p in DVE_PERIODS:
                    nc.vector.tensor_scalar(
                        out=ptt1.bitcast(I16),
                        in0=lg1,
                        scalar1=SCH_A,
                        scalar2=SCH_B,
                        op0=MULT,
                        op1=ADD,
                    )
                else:
                    nc.scalar.activation(ptt1, lg1, EXP, scale=0.125)
                nc.scalar.activation(ptt0, lg0, EXP, scale=0.125)
                # enqueue this chunk's PV work (h0 direct, h1 parked)
                gate = qb0_gate[p // 4] if qb == 0 else None
                for hf in (0, 1):
                    pv_deque.append(("pv", qb, 0, 2 * p + hf, ptt0, hf, gate))
                h1_parked.append(
                    [("pv", qb, 1, 2 * p + hf, ptt1, hf, gate) for hf in (0, 1)]
                )
                if p == NPER - 1:
                    # end of qb: tail0, then the parked h1 burst, tail1
                    pv_deque.append(("tail", qb, 0, None))
                    for ent in h1_parked:
                        pv_deque.extend(ent)
                    h1_parked.clear()
                    pv_deque.append(("tail", qb, 1, None))
                # drain ready PV work FIRST: the next QK pair's h1 matmul
                # waits on the lg1 ring semaphore, and emitting it at the
                # period head would block the ready PV stream behind it
                drain_pv(4, (qb, p))
                nxt = (qb, p + 1) if p + 1 < NPER else (qb + 1, 0)
                if nxt[0] < QB:
                    lg_cur = emit_qk_chunk(*nxt)
                drain_pv(3 if qb >= 1 else 2, (qb, p))
                for fn in hooks.pop((qb, p), []):
                    fn()
                drain_side(2)

        # ------------------------- drain tail ---------------------------------
        while pv_deque:
            drain_pv(1 << 30, None)
            drain_side(4)
        drain_side(1 << 30)
        assert not hooks, f"unconsumed hooks: {sorted(hooks)}"
        assert not pv_tiles, f"unfinished pv: {sorted(pv_tiles)}"


def build(enable_asserts=False):
    nc = bacc.Bacc(
        "TRN2",
        target_bir_lowering=False,
        debug=False,
        enable_asserts=enable_asserts,
        num_devices=N_CORES,
    )
    xq = nc.dram_tensor("xq", [4, P, S_FULL], F16, kind="ExternalInput").ap()
    xk = nc.dram_tensor("xk", [4, P, S_FULL], F16, kind="ExternalInput").ap()
    xv = nc.dram_tensor("xv", [4, P, S_FULL], F16, kind="ExternalInput").ap()
    wq = nc.dram_tensor("wq", [P, 4, GD], F16, kind="ExternalInput").ap()
    wk = nc.dram_tensor("wk", [P, 4, GD], F16, kind="ExternalInput").ap()
    wv = nc.dram_tensor("wv", [P, 4, GD], F16, kind="ExternalInput").ap()
    wo = nc.dram_tensor("wo", [GD, D], F16, kind="ExternalInput").ap()
    bq = nc.dram_tensor("bq", [GD], F32, kind="ExternalInput").ap()
    bk = nc.dram_tensor("bk", [GD], F32, kind="ExternalInput").ap()
    out = nc.dram_tensor("out", [S_FULL, D], F16, kind="ExternalOutput").ap()
    io = (xq, xk, xv, wq, wk, wv, wo, bq, bk, out)
    with tile.TileContext(nc) as tc:
        _emit(tc, io)
    nc.compile()
    return nc


def make_in_maps(queries, keys, values, Wq, bq, Wk, bk, Wv, bv, Wo, bo):
    f16 = lambda a: np.ascontiguousarray(
        np.asarray(a, dtype=np.float32).astype(np.float16)
    )
    f32 = lambda a: np.ascontiguousarray(np.asarray(a, dtype=np.float32))
    # pre-arrange projection weights to [P, 4, GD] (p t m) so the device DMA
    # reads contiguous lines
    warr = lambda a: np.ascontiguousarray(
        np.asarray(a, dtype=np.float32)
        .astype(np.float16)
        .reshape(4, P, -1)
        .transpose(1, 0, 2)
    )
    # host pre-transpose of the inputs: [S, D] -> [4, 128, S] (x.T tiled
    # over d) so the device never uses the slow xbar transpose
    xarr = lambda a: np.ascontiguousarray(
        np.asarray(a, dtype=np.float32).astype(np.float16).T.reshape(4, P, -1)
    )
    in_maps = []
    for c in range(N_CORES):
        b, g = divmod(c, 4)
        sl = slice(g * GD, (g + 1) * GD)
        in_maps.append(
            {
                "xq": xarr(queries[b]),
                "xk": xarr(keys[b]),
                "xv": xarr(values[b]),
                "wq": warr(np.asarray(Wq)[:, sl]),
                "wk": warr(np.asarray(Wk)[:, sl]),
                "wv": warr(np.asarray(Wv)[:, sl]),
                "wo": f16(np.asarray(Wo)[sl, :]),
                "bq": f32(np.asarray(bq)[sl]),
                "bk": f32(np.asarray(bk)[sl]),
            }
        )
    return in_maps


_NC = None
last_results = None


def kernel(queries, keys, values, Wq, bq, Wk, bk, Wv, bv, Wo, bo):
    global _NC, last_results
    import os
    if _NC is None:
        _NC = build()
    in_maps = make_in_maps(
        queries, keys, values, Wq, bq, Wk, bk, Wv, bv, Wo, bo
    )
    res = run_bass_kernel_spmd(
        _NC,
        in_maps,
        core_ids=list(range(N_CORES)),
        trace=bool(int(os.environ.get("MHA_TRACE", "0"))),
    )
    last_results = res
    outs = [
        np.asarray(res.results[c]["out"], dtype=np.float32)
        for c in range(N_CORES)
    ]
    full = np.empty((B_FULL, S_FULL, D), dtype=np.float32)
    # attention rows sum to 1, so the value bias contributes bv @ Wo to
    # every output row; fold it into the host-side constant with bo.
    bv16 = np.asarray(bv, np.float32).astype(np.float16).astype(np.float32)
    wo16 = np.asarray(Wo, np.float32).astype(np.float16).astype(np.float32)
    const = np.asarray(bo, dtype=np.float32) + bv16 @ wo16
    for b in range(B_FULL):
        full[b] = outs[4 * b] + outs[4 * b + 1] + outs[4 * b + 2] + outs[4 * b + 3]
        full[b] += const
    return full
